# revision 33
# baseline (speedup 1.0000x reference)
"""Trainium2 Bass kernel: single-head causal self-attention (fused streaming,
pair-split K/V projection with a software-pipelined AllGather exchange).

Reference computation (per batch b):
    Q = x @ Wq ; K = x @ Wk ; V = x @ Wv          (x: [S, D])
    S_sc = Q @ K^T / sqrt(D), causal masked
    out  = softmax(S_sc) @ V

Sharding: 8 cores, 4 batches -> core c handles batch b = c//2 and the
interleaved query half h = c%2 (q-tiles 2p+h, 1024 query rows). The K/V
projections for batch b are split across the pair by output column: core h
computes K^T/V for d_out columns [h*512, (h+1)*512) only (its Wk/Wv input
is the corresponding half of the weight matrix), and the halves are
exchanged with one combined pairwise AllGather per key-quarter through
DRAM bounce buffers. This halves the projection FLOPs vs computing K/V
redundantly.

Pipelined streaming structure: quarter qr projects its K^T/V half,
launches the AllGather, projects this core's Q^T strip, then runs the
attention strip for quarter qr-1 (whose gather already landed). The
attention strip for the last quarter runs after the loop. A strip's
causal extent is exactly the quarters processed up to it, so no masked
k-tile work is wasted beyond the 128-row tile granularity. K^T, V and
Q^T all stay resident in SBUF in bf16.

Scores are computed transposed (S^T[k, q]) so the softmax k-reduction
lands on partitions; row sums ride an extra ones-column matmul on the
same stationary P^T tiles. No max-subtraction (scores ~ N(0,1); exp is
safe in fp32). Causal mask built on device from a global-q-index input.
"""

import sys

try:
    import concourse.bass as bass  # noqa: F401
except ImportError:
    sys.path.insert(0, "/opt/trn_rl_repo")

import ml_dtypes
import numpy as np

import concourse.bass as bass
import concourse.tile as tile
from concourse import bacc, mybir
from concourse.bass_utils import run_bass_kernel_spmd

B, S, D = 4, 2048, 1024
NQ = 1024  # query rows per core
NK = 2048  # keys per core
P = 128
DT = D // P  # 8 d tiles
KT = NK // P  # 16 k tiles
W = 256  # queries per quarter-strip
NQR = 4  # key quarters (512 keys each)
HD = D // 2  # per-core K/V projection half (d_out columns)
HDT = DT // 2  # 4 d_out tiles per half
F32 = mybir.dt.float32
BF16 = mybir.dt.bfloat16
SCALE = 1.0 / np.sqrt(np.float32(D))  # 0.03125
BF_NP = ml_dtypes.bfloat16

_NC_CACHE = {}


def build_nc(n_cores=8):
    groups = [[2 * i, 2 * i + 1] for i in range(n_cores // 2)]
    nc = bacc.Bacc(None, target_bir_lowering=False, num_devices=n_cores)
    xkvT = nc.dram_tensor("xkvT", [D, NK], BF16, kind="ExternalInput")
    xqT = nc.dram_tensor("xqT", [D, NQ], BF16, kind="ExternalInput")
    qg = nc.dram_tensor("qg", [NQ], F32, kind="ExternalInput")
    wq_d = nc.dram_tensor("Wq", [D, D], BF16, kind="ExternalInput")
    wk_d = nc.dram_tensor("Wk", [D, HD], BF16, kind="ExternalInput")
    wv_d = nc.dram_tensor("Wv", [D, HD], BF16, kind="ExternalInput")
    out_d = nc.dram_tensor("out", [NQ, D], F32, kind="ExternalOutput")

    with tile.TileContext(nc) as tc:
        with (
            tc.tile_pool(name="persist", bufs=1) as persist,
            tc.tile_pool(name="misc", bufs=1) as misc,
        ):
            # Persistent SBUF residents (bf16): K^T [d, NK], V [k, D] per
            # k-tile, Q^T [d, NQ]
            kT = persist.tile([P, DT, NK], BF16, tag="kT")
            vT = persist.tile([P, KT, D], BF16, tag="vT")
            qT = persist.tile([P, DT, NQ], BF16, tag="qT")
            wk = persist.tile([P, DT, HD], BF16, tag="wk")
            wv = persist.tile([P, DT, HD], BF16, tag="wv")
            wq = persist.tile([P, DT, D], BF16, tag="wq")

            # Small constants: ones columns (moving operand of the row-sum
            # matmul), k-index vectors for the causal mask
            ones_f = misc.tile([P, 2], F32, tag="ones_f")
            nc.vector.memset(ones_f, 1.0)
            ones = misc.tile([P, 2], BF16, tag="ones")
            nc.vector.tensor_copy(ones, ones_f)
            pvec_i = misc.tile([P, 1], mybir.dt.int32, tag="pvec_i")
            nc.gpsimd.iota(pvec_i, pattern=[[0, 1]], base=0, channel_multiplier=1)
            pvec = misc.tile([P, 1], F32, tag="pvec")
            nc.vector.tensor_copy(pvec, pvec_i)
            kvecf = misc.tile([P, KT], F32, tag="kvecf")
            for kt in range(KT):
                nc.vector.tensor_scalar_add(kvecf[:, kt : kt + 1], pvec, float(kt * P))

            # Weight DMAs (scalar queue; xin/xq ride the sync queue in
            # parallel). wk first: quarter 0's K matmuls gate PE start.
            wk_t = wk_d.rearrange("(a p) o -> p a o", p=P)
            wv_t = wv_d.rearrange("(a p) o -> p a o", p=P)
            wq_t = wq_d.rearrange("(a p) o -> p a o", p=P)
            for di in range(DT):
                nc.scalar.dma_start(wk[:, di, :], wk_t[:, di, :])
            for di in range(DT):
                nc.scalar.dma_start(wv[:, di, :], wv_t[:, di, :])
            for di in range(DT):
                nc.scalar.dma_start(wq[:, di, :], wq_t[:, di, :])

            xkvT_t = xkvT.rearrange("(a p) s -> p a s", p=P)
            xqT_t = xqT.rearrange("(a p) s -> p a s", p=P)

            with (
                tc.tile_pool(name="xin", bufs=2) as xinp,
                tc.tile_pool(name="xq", bufs=2) as xqp,
                tc.tile_pool(name="stg", bufs=2) as stgp,
                tc.tile_pool(name="dram", bufs=2, space="DRAM") as dram,
                tc.tile_pool(name="sm", bufs=4) as sm,
                tc.tile_pool(name="pt", bufs=1) as ptp,
                tc.tile_pool(name="outp", bufs=2) as outp,
                tc.tile_pool(name="psA", bufs=2, space="PSUM") as psA,
                tc.tile_pool(name="psc", bufs=2, space="PSUM") as pscp,
                tc.tile_pool(name="psl", bufs=2, space="PSUM") as pslp,
            ):
                qgrids = {}
                kvouts = {}

                def readback(qs):
                    """Unpack gather qs into kT/vT. Emitted on the scalar
                    queue at the top of quarter qs+1: the queue stalls on
                    the collective here, but everything behind it (the exp
                    activations of attention strip qs) depends on this data
                    anyway."""
                    s0 = qs * 512
                    kv_t = kvouts.pop(qs).rearrange(
                        "(r b a p) s -> r b p a s", r=2, b=2, p=P
                    )
                    for r in range(2):
                        nc.scalar.dma_start(
                            kT[:, r * HDT : (r + 1) * HDT, s0 : s0 + 512], kv_t[r, 0]
                        )
                        nc.scalar.dma_start(
                            vT[:, qs * 4 : (qs + 1) * 4, r * HD : (r + 1) * HD],
                            kv_t[r, 1],
                        )

                def attn(qs):
                    """Attention strip qs: S^T -> exp -> mask -> P^T -> @V."""
                    q0 = qs * W
                    qgrid = qgrids.pop(qs)
                    # This strip holds global q-tiles 4qs+h and 4qs+2+h, so
                    # k-tiles >= ext_kt are fully masked and skipped.
                    ext_kt = 4 * (qs + 1)
                    pT = ptp.tile([P, KT, W], BF16, tag="pT", name="pT")
                    for kt in range(ext_kt):
                        ps = psA.tile([P, W], F32, tag="psA", name="ps")
                        for di in range(DT):
                            nc.tensor.matmul(
                                ps,
                                kT[:, di, kt * P : (kt + 1) * P],
                                qT[:, di, q0 : q0 + W],
                                start=(di == 0),
                                stop=(di == DT - 1),
                            )
                        et = sm.tile([P, W], F32, tag="et", name="et")
                        nc.scalar.activation(
                            et, ps, mybir.ActivationFunctionType.Exp, scale=float(SCALE)
                        )
                        mt = sm.tile([P, W], F32, tag="mt", name="mt")
                        nc.vector.tensor_scalar(
                            mt,
                            qgrid,
                            kvecf[:, kt : kt + 1],
                            None,
                            op0=mybir.AluOpType.is_ge,
                        )
                        nc.vector.tensor_mul(pT[:, kt, :], et, mt)

                    # context = P^T.T @ V (V resident in SBUF); row sums l
                    # ride a ones-column matmul on the same stationary P^T
                    ncq = W // P
                    cps = [
                        pscp.tile([P, D], F32, tag="psc", name=f"cps{i}")
                        for i in range(ncq)
                    ]
                    lps = [
                        pslp.tile([P, 2], F32, tag="psl", name=f"lps{i}")
                        for i in range(ncq)
                    ]
                    for kt in range(ext_kt):
                        for qt in range(ncq):
                            ej = 2 * (qs * ncq + qt) + 2  # this position's extent
                            if kt >= ej:
                                continue
                            lhs = pT[:, kt, qt * P : (qt + 1) * P]
                            nc.tensor.matmul(
                                cps[qt][:, 0:512],
                                lhs,
                                vT[:, kt, 0:512],
                                start=(kt == 0),
                                stop=(kt == ej - 1),
                            )
                            nc.tensor.matmul(
                                cps[qt][:, 512:1024],
                                lhs,
                                vT[:, kt, 512:1024],
                                start=(kt == 0),
                                stop=(kt == ej - 1),
                            )
                            nc.tensor.matmul(
                                lps[qt],
                                lhs,
                                ones,
                                start=(kt == 0),
                                stop=(kt == ej - 1),
                            )
                    for qt in range(ncq):
                        qrow = q0 + qt * P
                        rt = sm.tile([P, 1], F32, tag="rt", name="rt")
                        nc.vector.reciprocal(rt, lps[qt][:, 0:1])
                        ot = outp.tile([P, D], F32, tag="ot", name="ot")
                        nc.vector.tensor_scalar_mul(ot, cps[qt], rt)
                        nc.sync.dma_start(out_d[qrow : qrow + P, :], ot)

                for qr in range(NQR):
                    s0 = qr * 512
                    q0 = qr * W
                    # ---- input slices for this quarter (per-di DMAs for
                    # fine-grained matmul start) ----
                    xin = xinp.tile([P, DT, 512], BF16, tag="xin")
                    for di in range(DT):
                        nc.sync.dma_start(xin[:, di, :], xkvT_t[:, di, s0 : s0 + 512])
                    xq = xqp.tile([P, DT, W], BF16, tag="xq")
                    nc.sync.dma_start(xq, xqT_t[:, :, q0 : q0 + W])
                    # broadcast global q indices for this strip to all
                    # partitions (for the causal mask)
                    qgrid = sm.tile([P, W], F32, tag="qgrid")
                    qg_sl = qg[q0 : q0 + W]
                    nc.sync.dma_start(
                        qgrid,
                        bass.AP(
                            tensor=qg_sl.tensor,
                            offset=qg_sl.offset,
                            ap=[[0, P]] + list(qg_sl.ap),
                        ),
                    )
                    qgrids[qr] = qgrid

                    # ---- unpack the previous quarter's gather ----
                    if qr >= 1:
                        readback(qr - 1)

                    # ---- K^T half: out[d_out_half, k] accumulated over d_in ----
                    kvin_d = dram.tile([2 * HD, 512], BF16, tag="kvin")
                    ksg = stgp.tile([P, HDT, 512], BF16, tag="ksg")
                    for do in range(HDT):
                        ps = psA.tile([P, 512], F32, tag="psA")
                        for di in range(DT):
                            nc.tensor.matmul(
                                ps,
                                wk[:, di, do * P : (do + 1) * P],
                                xin[:, di, :],
                                start=(di == 0),
                                stop=(di == DT - 1),
                            )
                        nc.vector.tensor_copy(ksg[:, do, :], ps)
                    nc.sync.dma_start(
                        kvin_d[0:HD].rearrange("(a p) s -> p a s", p=P), ksg
                    )

                    # ---- V half: out[k, d_out_half] accumulated over d_in ----
                    vsg = stgp.tile([P, 4, HD], BF16, tag="vsg")
                    for st in range(4):
                        ps = psA.tile([P, 512], F32, tag="psA")
                        for di in range(DT):
                            nc.tensor.matmul(
                                ps,
                                xin[:, di, st * P : (st + 1) * P],
                                wv[:, di, :],
                                start=(di == 0),
                                stop=(di == DT - 1),
                            )
                        nc.vector.tensor_copy(vsg[:, st, :], ps)
                    nc.sync.dma_start(
                        kvin_d[HD : 2 * HD].rearrange("(a p) o -> p a o", p=P), vsg
                    )

                    # ---- combined pairwise AllGather: [myK|myV] x2 ranks ----
                    kvout_d = dram.tile([4 * HD, 512], BF16, tag="kvout")
                    nc.gpsimd.collective_compute(
                        "AllGather",
                        mybir.AluOpType.bypass,
                        replica_groups=groups,
                        ins=[kvin_d.opt()],
                        outs=[kvout_d.opt()],
                    )
                    kvouts[qr] = kvout_d

                    # ---- Q^T strip: out[d_out, q] accumulated over d_in ----
                    for do in range(DT):
                        ps = psA.tile([P, W], F32, tag="psA")
                        for di in range(DT):
                            nc.tensor.matmul(
                                ps,
                                wq[:, di, do * P : (do + 1) * P],
                                xq[:, di, :],
                                start=(di == 0),
                                stop=(di == DT - 1),
                            )
                        nc.vector.tensor_copy(qT[:, do, q0 : q0 + W], ps)

                    # ---- attention strip qr-1 (its gather has landed) ----
                    if qr >= 1:
                        attn(qr - 1)
                readback(NQR - 1)
                attn(NQR - 1)
    nc.compile()
    return nc


def _get_nc(key=8):
    if key not in _NC_CACHE:
        _NC_CACHE[key] = build_nc(n_cores=key if isinstance(key, int) else 8)
    return _NC_CACHE[key]


def _qsel(h):
    """Query rows for core-half h: global q-tiles h, 2+h, ..., 14+h.

    Position p's tile 2p+h needs only k < (2p+h+1)*128, letting the kernel
    skip fully-masked k-tiles at compile time with a core-uniform program."""
    tiles = np.arange(8) * 2 + h
    return (tiles[:, None] * P + np.arange(P)[None, :]).reshape(-1)


def make_in_maps(x, Wq, Wk, Wv, n_cores=8):
    x = np.asarray(x, dtype=np.float32)
    Wq = np.ascontiguousarray(np.asarray(Wq, dtype=np.float32)).astype(BF_NP)
    Wk = np.ascontiguousarray(np.asarray(Wk, dtype=np.float32)).astype(BF_NP)
    Wv = np.ascontiguousarray(np.asarray(Wv, dtype=np.float32)).astype(BF_NP)
    in_maps = []
    for c in range(n_cores):
        b, h = c // 2, c % 2
        qsel = _qsel(h)
        xbT = np.ascontiguousarray(x[b].T).astype(BF_NP)
        in_maps.append(
            {
                "xkvT": xbT,
                "xqT": np.ascontiguousarray(xbT[:, qsel]),
                "qg": qsel.astype(np.float32),
                "Wq": Wq,
                "Wk": np.ascontiguousarray(Wk[:, h * HD : (h + 1) * HD]),
                "Wv": np.ascontiguousarray(Wv[:, h * HD : (h + 1) * HD]),
            }
        )
    return in_maps


def kernel(x, Wq, Wk, Wv, _trace=False, _nc_key=8):
    nc = _get_nc(8)
    in_maps = make_in_maps(x, Wq, Wk, Wv)
    res = run_bass_kernel_spmd(nc, in_maps, core_ids=list(range(8)), trace=_trace)
    out = np.empty((B, S, D), dtype=np.float32)
    for c in range(8):
        b, h = c // 2, c % 2
        out[b, _qsel(h), :] = res.results[c]["out"]
    if _trace:
        kernel.last_results = res
    return out


# revision 39
# speedup vs baseline: 1.0351x; 1.0351x over previous
"""Trainium2 Bass kernel: single-head causal self-attention (fused streaming,
pair-split K/V projection with a software-pipelined AllGather exchange).

Reference computation (per batch b):
    Q = x @ Wq ; K = x @ Wk ; V = x @ Wv          (x: [S, D])
    S_sc = Q @ K^T / sqrt(D), causal masked
    out  = softmax(S_sc) @ V

Sharding: 8 cores, 4 batches -> core c handles batch b = c//2 and the
interleaved query half h = c%2 (q-tiles 2p+h, 1024 query rows). The K/V
projections for batch b are split across the pair by output column: core h
computes K^T/V for d_out columns [h*512, (h+1)*512) only (its Wk/Wv input
is the corresponding half of the weight matrix), and the halves are
exchanged with one combined pairwise AllGather per key-quarter through
DRAM bounce buffers. This halves the projection FLOPs vs computing K/V
redundantly.

Pipelined streaming structure: quarter qr projects its K^T/V half,
launches the AllGather, projects this core's Q^T strip, then runs the
attention strip for quarter qr-1 (whose gather already landed). The
attention strip for the last quarter runs after the loop. A strip's
causal extent is exactly the quarters processed up to it, so no masked
k-tile work is wasted beyond the 128-row tile granularity. K^T, V and
Q^T all stay resident in SBUF in bf16.

Scores are computed transposed (S^T[k, q]) so the softmax k-reduction
lands on partitions; row sums ride an extra ones-column matmul on the
same stationary P^T tiles. No max-subtraction (scores ~ N(0,1); exp is
safe in fp32). Causal mask built on device from a global-q-index input.
"""

import sys

try:
    import concourse.bass as bass  # noqa: F401
except ImportError:
    sys.path.insert(0, "/opt/trn_rl_repo")

import ml_dtypes
import numpy as np

import concourse.bass as bass
import concourse.tile as tile
from concourse import bacc, mybir
from concourse.bass_utils import run_bass_kernel_spmd

B, S, D = 4, 2048, 1024
NQ = 1024  # query rows per core
NK = 2048  # keys per core
P = 128
DT = D // P  # 8 d tiles
KT = NK // P  # 16 k tiles
W = 256  # queries per quarter-strip
NQR = 4  # key quarters (512 keys each)
HD = D // 2  # per-core K/V projection half (d_out columns)
HDT = DT // 2  # 4 d_out tiles per half
F32 = mybir.dt.float32
BF16 = mybir.dt.bfloat16
SCALE = 1.0 / np.sqrt(np.float32(D))  # 0.03125
BF_NP = ml_dtypes.bfloat16

_NC_CACHE = {}


def build_nc(n_cores=8):
    groups = [[2 * i, 2 * i + 1] for i in range(n_cores // 2)]
    nc = bacc.Bacc(None, target_bir_lowering=False, num_devices=n_cores)
    xkvT = nc.dram_tensor("xkvT", [D, NK], BF16, kind="ExternalInput")
    xqT = nc.dram_tensor("xqT", [D, NQ], BF16, kind="ExternalInput")
    qg = nc.dram_tensor("qg", [NQ], F32, kind="ExternalInput")
    wq_d = nc.dram_tensor("Wq", [D, D], BF16, kind="ExternalInput")
    wk_d = nc.dram_tensor("Wk", [D, HD], BF16, kind="ExternalInput")
    wv_d = nc.dram_tensor("Wv", [D, HD], BF16, kind="ExternalInput")
    out_d = nc.dram_tensor("out", [NQ, D], F32, kind="ExternalOutput")

    with tile.TileContext(nc) as tc:
        with (
            tc.tile_pool(name="persist", bufs=1) as persist,
            tc.tile_pool(name="misc", bufs=1) as misc,
        ):
            # Persistent SBUF residents (bf16): K^T [d, NK], V [k, D] per
            # k-tile, Q^T [d, NQ]
            kT = persist.tile([P, DT, NK], BF16, tag="kT")
            vT = persist.tile([P, KT, D], BF16, tag="vT")
            qT = persist.tile([P, DT, NQ], BF16, tag="qT")
            wk = persist.tile([P, DT, HD], BF16, tag="wk")
            wv = persist.tile([P, DT, HD], BF16, tag="wv")
            wq = persist.tile([P, DT, D], BF16, tag="wq")

            # Small constants: ones columns (moving operand of the row-sum
            # matmul), k-index vectors for the causal mask
            ones_f = misc.tile([P, 2], F32, tag="ones_f")
            nc.vector.memset(ones_f, 1.0)
            ones = misc.tile([P, 2], BF16, tag="ones")
            nc.vector.tensor_copy(ones, ones_f)
            pvec_i = misc.tile([P, 1], mybir.dt.int32, tag="pvec_i")
            nc.gpsimd.iota(pvec_i, pattern=[[0, 1]], base=0, channel_multiplier=1)
            pvec = misc.tile([P, 1], F32, tag="pvec")
            nc.vector.tensor_copy(pvec, pvec_i)
            kvecf = misc.tile([P, KT], F32, tag="kvecf")
            for kt in range(KT):
                nc.vector.tensor_scalar_add(kvecf[:, kt : kt + 1], pvec, float(kt * P))

            # Weight DMAs (scalar queue; xin/xq ride the sync queue in
            # parallel). wk first: quarter 0's K matmuls gate PE start.
            wk_t = wk_d.rearrange("(a p) o -> p a o", p=P)
            wv_t = wv_d.rearrange("(a p) o -> p a o", p=P)
            wq_t = wq_d.rearrange("(a p) o -> p a o", p=P)
            for di in range(DT):
                nc.scalar.dma_start(wk[:, di, :], wk_t[:, di, :])
            for di in range(DT):
                nc.scalar.dma_start(wv[:, di, :], wv_t[:, di, :])
            for di in range(DT):
                nc.scalar.dma_start(wq[:, di, :], wq_t[:, di, :])

            xkvT_t = xkvT.rearrange("(a p) s -> p a s", p=P)
            xqT_t = xqT.rearrange("(a p) s -> p a s", p=P)

            with (
                tc.tile_pool(name="xin", bufs=2) as xinp,
                tc.tile_pool(name="xq", bufs=2) as xqp,
                tc.tile_pool(name="stg", bufs=2) as stgp,
                tc.tile_pool(name="dram", bufs=2, space="DRAM") as dram,
                tc.tile_pool(name="sm", bufs=4) as sm,
                tc.tile_pool(name="pt", bufs=1) as ptp,
                tc.tile_pool(name="outp", bufs=2) as outp,
                tc.tile_pool(name="psA", bufs=2, space="PSUM") as psA,
                tc.tile_pool(name="psc", bufs=2, space="PSUM") as pscp,
                tc.tile_pool(name="psl", bufs=2, space="PSUM") as pslp,
            ):
                qgrids = {}
                kvouts = {}

                def readback(qs):
                    """Unpack gather qs into kT/vT. Emitted on the scalar
                    queue at the top of quarter qs+1: the queue stalls on
                    the collective here, but everything behind it (the exp
                    activations of attention strip qs) depends on this data
                    anyway."""
                    s0 = qs * 512
                    kv_t = kvouts.pop(qs).rearrange(
                        "(r b a p) s -> r b p a s", r=2, b=2, p=P
                    )
                    for r in range(2):
                        nc.scalar.dma_start(
                            kT[:, r * HDT : (r + 1) * HDT, s0 : s0 + 512], kv_t[r, 0]
                        )
                        nc.scalar.dma_start(
                            vT[:, qs * 4 : (qs + 1) * 4, r * HD : (r + 1) * HD],
                            kv_t[r, 1],
                        )

                strips = {}

                def attn_part(qs, kt_lo, kt_hi, finalize):
                    """Attention strip qs for k-tiles [kt_lo, kt_hi):
                    S^T -> exp -> mask -> P^T -> @V (+ finalize).

                    Split so the k-tiles of older quarters (which need no
                    fresh gather) run early, and only the 4 diagonal
                    k-tiles wait on quarter qs's own gather."""
                    q0 = qs * W
                    ncq = W // P
                    if qs not in strips:
                        strips[qs] = (
                            ptp.tile([P, KT, W], BF16, tag="pT", name="pT"),
                            [
                                pscp.tile([P, D], F32, tag="psc", name=f"cps{i}")
                                for i in range(ncq)
                            ],
                            [
                                pslp.tile([P, 2], F32, tag="psl", name=f"lps{i}")
                                for i in range(ncq)
                            ],
                            qgrids.pop(qs),
                        )
                    pT, cps, lps, qgrid = strips[qs]
                    for kt in range(kt_lo, kt_hi):
                        ps = psA.tile([P, W], F32, tag="psA", name="ps")
                        for di in range(DT):
                            nc.tensor.matmul(
                                ps,
                                kT[:, di, kt * P : (kt + 1) * P],
                                qT[:, di, q0 : q0 + W],
                                start=(di == 0),
                                stop=(di == DT - 1),
                            )
                        et = sm.tile([P, W], F32, tag="et", name="et")
                        nc.scalar.activation(
                            et, ps, mybir.ActivationFunctionType.Exp, scale=float(SCALE)
                        )
                        mt = sm.tile([P, W], F32, tag="mt", name="mt")
                        nc.vector.tensor_scalar(
                            mt,
                            qgrid,
                            kvecf[:, kt : kt + 1],
                            None,
                            op0=mybir.AluOpType.is_ge,
                        )
                        nc.vector.tensor_mul(pT[:, kt, :], et, mt)

                    # context = P^T.T @ V (V resident in SBUF); row sums l
                    # ride a ones-column matmul on the same stationary P^T
                    for kt in range(kt_lo, kt_hi):
                        for qt in range(ncq):
                            ej = 2 * (qs * ncq + qt) + 2  # this position's extent
                            if kt >= ej:
                                continue
                            lhs = pT[:, kt, qt * P : (qt + 1) * P]
                            nc.tensor.matmul(
                                cps[qt][:, 0:512],
                                lhs,
                                vT[:, kt, 0:512],
                                start=(kt == 0),
                                stop=(kt == ej - 1),
                            )
                            nc.tensor.matmul(
                                cps[qt][:, 512:1024],
                                lhs,
                                vT[:, kt, 512:1024],
                                start=(kt == 0),
                                stop=(kt == ej - 1),
                            )
                            nc.tensor.matmul(
                                lps[qt],
                                lhs,
                                ones,
                                start=(kt == 0),
                                stop=(kt == ej - 1),
                            )
                    if not finalize:
                        return
                    del strips[qs]
                    for qt in range(ncq):
                        qrow = q0 + qt * P
                        rt = sm.tile([P, 1], F32, tag="rt", name="rt")
                        nc.vector.reciprocal(rt, lps[qt][:, 0:1])
                        ot = outp.tile([P, D], F32, tag="ot", name="ot")
                        nc.vector.tensor_scalar_mul(ot, cps[qt], rt)
                        nc.sync.dma_start(out_d[qrow : qrow + P, :], ot)

                for qr in range(NQR):
                    s0 = qr * 512
                    q0 = qr * W
                    # ---- input slices for this quarter (per-di DMAs for
                    # fine-grained matmul start) ----
                    xin = xinp.tile([P, DT, 512], BF16, tag="xin")
                    for di in range(DT):
                        nc.sync.dma_start(xin[:, di, :], xkvT_t[:, di, s0 : s0 + 512])
                    xq = xqp.tile([P, DT, W], BF16, tag="xq")
                    nc.sync.dma_start(xq, xqT_t[:, :, q0 : q0 + W])
                    # broadcast global q indices for this strip to all
                    # partitions (for the causal mask)
                    qgrid = sm.tile([P, W], F32, tag="qgrid")
                    qg_sl = qg[q0 : q0 + W]
                    nc.sync.dma_start(
                        qgrid,
                        bass.AP(
                            tensor=qg_sl.tensor,
                            offset=qg_sl.offset,
                            ap=[[0, P]] + list(qg_sl.ap),
                        ),
                    )
                    qgrids[qr] = qgrid

                    # ---- K^T half: out[d_out_half, k] accumulated over d_in ----
                    kvin_d = dram.tile([2 * HD, 512], BF16, tag="kvin")
                    ksg = stgp.tile([P, HDT, 512], BF16, tag="ksg")
                    for do in range(HDT):
                        ps = psA.tile([P, 512], F32, tag="psA")
                        for di in range(DT):
                            nc.tensor.matmul(
                                ps,
                                wk[:, di, do * P : (do + 1) * P],
                                xin[:, di, :],
                                start=(di == 0),
                                stop=(di == DT - 1),
                            )
                        nc.vector.tensor_copy(ksg[:, do, :], ps)
                    nc.sync.dma_start(
                        kvin_d[0:HD].rearrange("(a p) s -> p a s", p=P), ksg
                    )

                    # ---- V half: out[k, d_out_half] accumulated over d_in ----
                    vsg = stgp.tile([P, 4, HD], BF16, tag="vsg")
                    for st in range(4):
                        ps = psA.tile([P, 512], F32, tag="psA")
                        for di in range(DT):
                            nc.tensor.matmul(
                                ps,
                                xin[:, di, st * P : (st + 1) * P],
                                wv[:, di, :],
                                start=(di == 0),
                                stop=(di == DT - 1),
                            )
                        nc.vector.tensor_copy(vsg[:, st, :], ps)
                    nc.sync.dma_start(
                        kvin_d[HD : 2 * HD].rearrange("(a p) o -> p a o", p=P), vsg
                    )

                    # ---- combined pairwise AllGather: [myK|myV] x2 ranks ----
                    kvout_d = dram.tile([4 * HD, 512], BF16, tag="kvout")
                    nc.gpsimd.collective_compute(
                        "AllGather",
                        mybir.AluOpType.bypass,
                        replica_groups=groups,
                        ins=[kvin_d.opt()],
                        outs=[kvout_d.opt()],
                    )
                    kvouts[qr] = kvout_d

                    # ---- strip qr-1, old k-tiles (no fresh gather needed);
                    # then unpack quarter qr-1's gather (the scalar-queue
                    # wait here sits after part-1's exps, before part-2's) ----
                    if qr >= 1:
                        attn_part(qr - 1, 0, 4 * (qr - 1), finalize=False)
                        readback(qr - 1)

                    # ---- Q^T strip: out[d_out, q] accumulated over d_in ----
                    for do in range(DT):
                        ps = psA.tile([P, W], F32, tag="psA")
                        for di in range(DT):
                            nc.tensor.matmul(
                                ps,
                                wq[:, di, do * P : (do + 1) * P],
                                xq[:, di, :],
                                start=(di == 0),
                                stop=(di == DT - 1),
                            )
                        nc.vector.tensor_copy(qT[:, do, q0 : q0 + W], ps)

                    # ---- strip qr-1, diagonal k-tiles (gather landed) ----
                    if qr >= 1:
                        attn_part(qr - 1, 4 * (qr - 1), 4 * qr, finalize=True)
                qs = NQR - 1
                attn_part(qs, 0, 4 * qs, finalize=False)
                readback(qs)
                attn_part(qs, 4 * qs, 4 * (qs + 1), finalize=True)
    nc.compile()
    return nc


def _get_nc(key=8):
    if key not in _NC_CACHE:
        _NC_CACHE[key] = build_nc(n_cores=key if isinstance(key, int) else 8)
    return _NC_CACHE[key]


def _qsel(h):
    """Query rows for core-half h: global q-tiles h, 2+h, ..., 14+h.

    Position p's tile 2p+h needs only k < (2p+h+1)*128, letting the kernel
    skip fully-masked k-tiles at compile time with a core-uniform program."""
    tiles = np.arange(8) * 2 + h
    return (tiles[:, None] * P + np.arange(P)[None, :]).reshape(-1)


def make_in_maps(x, Wq, Wk, Wv, n_cores=8):
    x = np.asarray(x, dtype=np.float32)
    Wq = np.ascontiguousarray(np.asarray(Wq, dtype=np.float32)).astype(BF_NP)
    Wk = np.ascontiguousarray(np.asarray(Wk, dtype=np.float32)).astype(BF_NP)
    Wv = np.ascontiguousarray(np.asarray(Wv, dtype=np.float32)).astype(BF_NP)
    in_maps = []
    for c in range(n_cores):
        b, h = c // 2, c % 2
        qsel = _qsel(h)
        xbT = np.ascontiguousarray(x[b].T).astype(BF_NP)
        in_maps.append(
            {
                "xkvT": xbT,
                "xqT": np.ascontiguousarray(xbT[:, qsel]),
                "qg": qsel.astype(np.float32),
                "Wq": Wq,
                "Wk": np.ascontiguousarray(Wk[:, h * HD : (h + 1) * HD]),
                "Wv": np.ascontiguousarray(Wv[:, h * HD : (h + 1) * HD]),
            }
        )
    return in_maps


def kernel(x, Wq, Wk, Wv, _trace=False, _nc_key=8):
    nc = _get_nc(8)
    in_maps = make_in_maps(x, Wq, Wk, Wv)
    res = run_bass_kernel_spmd(nc, in_maps, core_ids=list(range(8)), trace=_trace)
    out = np.empty((B, S, D), dtype=np.float32)
    for c in range(8):
        b, h = c // 2, c % 2
        out[b, _qsel(h), :] = res.results[c]["out"]
    if _trace:
        kernel.last_results = res
    return out


# revision 40
# speedup vs baseline: 1.0524x; 1.0167x over previous
"""Trainium2 Bass kernel: single-head causal self-attention (fused streaming,
pair-split K/V projection with a software-pipelined AllGather exchange).

Reference computation (per batch b):
    Q = x @ Wq ; K = x @ Wk ; V = x @ Wv          (x: [S, D])
    S_sc = Q @ K^T / sqrt(D), causal masked
    out  = softmax(S_sc) @ V

Sharding: 8 cores, 4 batches -> core c handles batch b = c//2 and the
interleaved query half h = c%2 (q-tiles 2p+h, 1024 query rows). The K/V
projections for batch b are split across the pair by output column: core h
computes K^T/V for d_out columns [h*512, (h+1)*512) only (its Wk/Wv input
is the corresponding half of the weight matrix), and the halves are
exchanged with one combined pairwise AllGather per key-quarter through
DRAM bounce buffers. This halves the projection FLOPs vs computing K/V
redundantly.

Pipelined streaming structure: quarter qr projects its K^T/V half,
launches the AllGather, projects this core's Q^T strip, then runs the
attention strip for quarter qr-1 (whose gather already landed). The
attention strip for the last quarter runs after the loop. A strip's
causal extent is exactly the quarters processed up to it, so no masked
k-tile work is wasted beyond the 128-row tile granularity. K^T, V and
Q^T all stay resident in SBUF in bf16.

Scores are computed transposed (S^T[k, q]) so the softmax k-reduction
lands on partitions; row sums ride an extra ones-column matmul on the
same stationary P^T tiles. No max-subtraction (scores ~ N(0,1); exp is
safe in fp32). Causal mask built on device from a global-q-index input.
"""

import sys

try:
    import concourse.bass as bass  # noqa: F401
except ImportError:
    sys.path.insert(0, "/opt/trn_rl_repo")

import ml_dtypes
import numpy as np

import concourse.bass as bass
import concourse.tile as tile
from concourse import bacc, mybir
from concourse.bass_utils import run_bass_kernel_spmd

B, S, D = 4, 2048, 1024
NQ = 1024  # query rows per core
NK = 2048  # keys per core
P = 128
DT = D // P  # 8 d tiles
KT = NK // P  # 16 k tiles
W = 256  # queries per quarter-strip
NQR = 4  # key quarters (512 keys each)
HD = D // 2  # per-core K/V projection half (d_out columns)
HDT = DT // 2  # 4 d_out tiles per half
F32 = mybir.dt.float32
BF16 = mybir.dt.bfloat16
SCALE = 1.0 / np.sqrt(np.float32(D))  # 0.03125
BF_NP = ml_dtypes.bfloat16

_NC_CACHE = {}


def build_nc(n_cores=8):
    groups = [[2 * i, 2 * i + 1] for i in range(n_cores // 2)]
    nc = bacc.Bacc(None, target_bir_lowering=False, num_devices=n_cores)
    xkvT = nc.dram_tensor("xkvT", [D, NK], BF16, kind="ExternalInput")
    xqT = nc.dram_tensor("xqT", [D, NQ], BF16, kind="ExternalInput")
    qg = nc.dram_tensor("qg", [NQ], F32, kind="ExternalInput")
    wq_d = nc.dram_tensor("Wq", [D, D], BF16, kind="ExternalInput")
    wk_d = nc.dram_tensor("Wk", [D, HD], BF16, kind="ExternalInput")
    wv_d = nc.dram_tensor("Wv", [D, HD], BF16, kind="ExternalInput")
    out_d = nc.dram_tensor("out", [NQ, D], F32, kind="ExternalOutput")

    with tile.TileContext(nc) as tc:
        with (
            tc.tile_pool(name="persist", bufs=1) as persist,
            tc.tile_pool(name="misc", bufs=1) as misc,
        ):
            # Persistent SBUF residents (bf16): K^T [d, NK], V [k, D] per
            # k-tile, Q^T [d, NQ]
            kT = persist.tile([P, DT, NK], BF16, tag="kT")
            vT = persist.tile([P, KT, D], BF16, tag="vT")
            qT = persist.tile([P, DT, NQ], BF16, tag="qT")
            wk = persist.tile([P, DT, HD], BF16, tag="wk")
            wv = persist.tile([P, DT, HD], BF16, tag="wv")
            wq = persist.tile([P, DT, D], BF16, tag="wq")

            # Small constants: ones columns (moving operand of the row-sum
            # matmul), k-index vectors for the causal mask
            ones_f = misc.tile([P, 2], F32, tag="ones_f")
            nc.vector.memset(ones_f, 1.0)
            ones = misc.tile([P, 2], BF16, tag="ones")
            nc.vector.tensor_copy(ones, ones_f)
            pvec_i = misc.tile([P, 1], mybir.dt.int32, tag="pvec_i")
            nc.gpsimd.iota(pvec_i, pattern=[[0, 1]], base=0, channel_multiplier=1)
            pvec = misc.tile([P, 1], F32, tag="pvec")
            nc.vector.tensor_copy(pvec, pvec_i)
            kvecf = misc.tile([P, KT], F32, tag="kvecf")
            for kt in range(KT):
                nc.vector.tensor_scalar_add(kvecf[:, kt : kt + 1], pvec, float(kt * P))

            # Weight DMAs (scalar queue; xin/xq ride the sync queue in
            # parallel). wk first: quarter 0's K matmuls gate PE start.
            wk_t = wk_d.rearrange("(a p) o -> p a o", p=P)
            wv_t = wv_d.rearrange("(a p) o -> p a o", p=P)
            wq_t = wq_d.rearrange("(a p) o -> p a o", p=P)
            for di in range(DT):
                nc.scalar.dma_start(wk[:, di, :], wk_t[:, di, :])
            for di in range(DT):
                nc.scalar.dma_start(wv[:, di, :], wv_t[:, di, :])
            for di in range(DT):
                nc.scalar.dma_start(wq[:, di, :], wq_t[:, di, :])

            xkvT_t = xkvT.rearrange("(a p) s -> p a s", p=P)
            xqT_t = xqT.rearrange("(a p) s -> p a s", p=P)

            with (
                tc.tile_pool(name="xin", bufs=2) as xinp,
                tc.tile_pool(name="xq", bufs=2) as xqp,
                tc.tile_pool(name="stg", bufs=2) as stgp,
                tc.tile_pool(name="dram", bufs=2, space="DRAM") as dram,
                tc.tile_pool(name="sm", bufs=4) as sm,
                tc.tile_pool(name="pt", bufs=1) as ptp,
                tc.tile_pool(name="outp", bufs=2) as outp,
                tc.tile_pool(name="psA", bufs=2, space="PSUM") as psA,
                tc.tile_pool(name="psc", bufs=2, space="PSUM") as pscp,
                tc.tile_pool(name="psl", bufs=2, space="PSUM") as pslp,
            ):
                qgrids = {}
                kvouts = {}

                def readback(qs):
                    """Unpack gather qs into kT/vT. Emitted on the scalar
                    queue at the top of quarter qs+1: the queue stalls on
                    the collective here, but everything behind it (the exp
                    activations of attention strip qs) depends on this data
                    anyway."""
                    s0 = qs * 512
                    kv_t = kvouts.pop(qs).rearrange(
                        "(r b a p) s -> r b p a s", r=2, b=2, p=P
                    )
                    for r in range(2):
                        nc.scalar.dma_start(
                            kT[:, r * HDT : (r + 1) * HDT, s0 : s0 + 512], kv_t[r, 0]
                        )
                        nc.scalar.dma_start(
                            vT[:, qs * 4 : (qs + 1) * 4, r * HD : (r + 1) * HD],
                            kv_t[r, 1],
                        )

                def attn(qs):
                    """Attention strip qs: S^T -> exp -> mask -> P^T -> @V."""
                    q0 = qs * W
                    qgrid = qgrids.pop(qs)
                    # This strip holds global q-tiles 4qs+h and 4qs+2+h, so
                    # k-tiles >= ext_kt are fully masked and skipped.
                    ext_kt = 4 * (qs + 1)
                    pT = ptp.tile([P, KT, W], BF16, tag="pT", name="pT")
                    for kt in range(ext_kt):
                        ps = psA.tile([P, W], F32, tag="psA", name="ps")
                        for di in range(DT):
                            nc.tensor.matmul(
                                ps,
                                kT[:, di, kt * P : (kt + 1) * P],
                                qT[:, di, q0 : q0 + W],
                                start=(di == 0),
                                stop=(di == DT - 1),
                            )
                        et = sm.tile([P, W], F32, tag="et", name="et")
                        nc.scalar.activation(
                            et, ps, mybir.ActivationFunctionType.Exp, scale=float(SCALE)
                        )
                        mt = sm.tile([P, W], F32, tag="mt", name="mt")
                        nc.vector.tensor_scalar(
                            mt,
                            qgrid,
                            kvecf[:, kt : kt + 1],
                            None,
                            op0=mybir.AluOpType.is_ge,
                        )
                        nc.vector.tensor_mul(pT[:, kt, :], et, mt)

                    # context = P^T.T @ V (V resident in SBUF); row sums l
                    # ride a ones-column matmul on the same stationary P^T
                    ncq = W // P
                    cps = [
                        pscp.tile([P, D], F32, tag="psc", name=f"cps{i}")
                        for i in range(ncq)
                    ]
                    lps = [
                        pslp.tile([P, 2], F32, tag="psl", name=f"lps{i}")
                        for i in range(ncq)
                    ]
                    for kt in range(ext_kt):
                        for qt in range(ncq):
                            ej = 2 * (qs * ncq + qt) + 2  # this position's extent
                            if kt >= ej:
                                continue
                            lhs = pT[:, kt, qt * P : (qt + 1) * P]
                            nc.tensor.matmul(
                                cps[qt][:, 0:512],
                                lhs,
                                vT[:, kt, 0:512],
                                start=(kt == 0),
                                stop=(kt == ej - 1),
                            )
                            nc.tensor.matmul(
                                cps[qt][:, 512:1024],
                                lhs,
                                vT[:, kt, 512:1024],
                                start=(kt == 0),
                                stop=(kt == ej - 1),
                            )
                            nc.tensor.matmul(
                                lps[qt],
                                lhs,
                                ones,
                                start=(kt == 0),
                                stop=(kt == ej - 1),
                            )
                    for qt in range(ncq):
                        qrow = q0 + qt * P
                        rt = sm.tile([P, 1], F32, tag="rt", name="rt")
                        nc.vector.reciprocal(rt, lps[qt][:, 0:1])
                        ot = outp.tile([P, D], F32, tag="ot", name="ot")
                        nc.vector.tensor_scalar_mul(ot, cps[qt], rt)
                        nc.sync.dma_start(out_d[qrow : qrow + P, :], ot)

                for qr in range(NQR):
                    s0 = qr * 512
                    q0 = qr * W
                    # ---- input slices for this quarter (per-di DMAs for
                    # fine-grained matmul start) ----
                    xin = xinp.tile([P, DT, 512], BF16, tag="xin")
                    for di in range(DT):
                        nc.sync.dma_start(xin[:, di, :], xkvT_t[:, di, s0 : s0 + 512])
                    xq = xqp.tile([P, DT, W], BF16, tag="xq")
                    nc.sync.dma_start(xq, xqT_t[:, :, q0 : q0 + W])
                    # broadcast global q indices for this strip to all
                    # partitions (for the causal mask)
                    qgrid = sm.tile([P, W], F32, tag="qgrid")
                    qg_sl = qg[q0 : q0 + W]
                    nc.sync.dma_start(
                        qgrid,
                        bass.AP(
                            tensor=qg_sl.tensor,
                            offset=qg_sl.offset,
                            ap=[[0, P]] + list(qg_sl.ap),
                        ),
                    )
                    qgrids[qr] = qgrid

                    # ---- unpack the previous quarter's gather ----
                    if qr >= 1:
                        readback(qr - 1)

                    # ---- K^T half: out[d_out_half, k] accumulated over d_in ----
                    kvin_d = dram.tile([2 * HD, 512], BF16, tag="kvin")
                    ksg = stgp.tile([P, HDT, 512], BF16, tag="ksg")
                    for do in range(HDT):
                        ps = psA.tile([P, 512], F32, tag="psA")
                        for di in range(DT):
                            nc.tensor.matmul(
                                ps,
                                wk[:, di, do * P : (do + 1) * P],
                                xin[:, di, :],
                                start=(di == 0),
                                stop=(di == DT - 1),
                            )
                        nc.vector.tensor_copy(ksg[:, do, :], ps)
                    nc.sync.dma_start(
                        kvin_d[0:HD].rearrange("(a p) s -> p a s", p=P), ksg
                    )

                    # ---- V half: out[k, d_out_half] accumulated over d_in ----
                    vsg = stgp.tile([P, 4, HD], BF16, tag="vsg")
                    for st in range(4):
                        ps = psA.tile([P, 512], F32, tag="psA")
                        for di in range(DT):
                            nc.tensor.matmul(
                                ps,
                                xin[:, di, st * P : (st + 1) * P],
                                wv[:, di, :],
                                start=(di == 0),
                                stop=(di == DT - 1),
                            )
                        nc.vector.tensor_copy(vsg[:, st, :], ps)
                    nc.sync.dma_start(
                        kvin_d[HD : 2 * HD].rearrange("(a p) o -> p a o", p=P), vsg
                    )

                    # ---- combined pairwise AllGather: [myK|myV] x2 ranks ----
                    kvout_d = dram.tile([4 * HD, 512], BF16, tag="kvout")
                    nc.gpsimd.collective_compute(
                        "AllGather",
                        mybir.AluOpType.bypass,
                        replica_groups=groups,
                        ins=[kvin_d.opt()],
                        outs=[kvout_d.opt()],
                    )
                    kvouts[qr] = kvout_d

                    # ---- Q^T strip: out[d_out, q] accumulated over d_in ----
                    for do in range(DT):
                        ps = psA.tile([P, W], F32, tag="psA")
                        for di in range(DT):
                            nc.tensor.matmul(
                                ps,
                                wq[:, di, do * P : (do + 1) * P],
                                xq[:, di, :],
                                start=(di == 0),
                                stop=(di == DT - 1),
                            )
                        nc.vector.tensor_copy(qT[:, do, q0 : q0 + W], ps)

                    # ---- attention strip qr-1 (its gather has landed) ----
                    if qr >= 1:
                        attn(qr - 1)
                readback(NQR - 1)
                attn(NQR - 1)
    nc.compile()
    return nc


def _get_nc(key=8):
    if key not in _NC_CACHE:
        _NC_CACHE[key] = build_nc(n_cores=key if isinstance(key, int) else 8)
    return _NC_CACHE[key]


def _qsel(h):
    """Query rows for core-half h: global q-tiles h, 2+h, ..., 14+h.

    Position p's tile 2p+h needs only k < (2p+h+1)*128, letting the kernel
    skip fully-masked k-tiles at compile time with a core-uniform program."""
    tiles = np.arange(8) * 2 + h
    return (tiles[:, None] * P + np.arange(P)[None, :]).reshape(-1)


def make_in_maps(x, Wq, Wk, Wv, n_cores=8):
    x = np.asarray(x, dtype=np.float32)
    Wq = np.ascontiguousarray(np.asarray(Wq, dtype=np.float32)).astype(BF_NP)
    Wk = np.ascontiguousarray(np.asarray(Wk, dtype=np.float32)).astype(BF_NP)
    Wv = np.ascontiguousarray(np.asarray(Wv, dtype=np.float32)).astype(BF_NP)
    in_maps = []
    for c in range(n_cores):
        b, h = c // 2, c % 2
        qsel = _qsel(h)
        xbT = np.ascontiguousarray(x[b].T).astype(BF_NP)
        in_maps.append(
            {
                "xkvT": xbT,
                "xqT": np.ascontiguousarray(xbT[:, qsel]),
                "qg": qsel.astype(np.float32),
                "Wq": Wq,
                "Wk": np.ascontiguousarray(Wk[:, h * HD : (h + 1) * HD]),
                "Wv": np.ascontiguousarray(Wv[:, h * HD : (h + 1) * HD]),
            }
        )
    return in_maps


def kernel(x, Wq, Wk, Wv, _trace=False, _nc_key=8):
    nc = _get_nc(8)
    in_maps = make_in_maps(x, Wq, Wk, Wv)
    res = run_bass_kernel_spmd(nc, in_maps, core_ids=list(range(8)), trace=_trace)
    out = np.empty((B, S, D), dtype=np.float32)
    for c in range(8):
        b, h = c // 2, c % 2
        out[b, _qsel(h), :] = res.results[c]["out"]
    if _trace:
        kernel.last_results = res
    return out
